# revision 7
# baseline (speedup 1.0000x reference)
import sys
import types

sys.path.insert(0, "/opt/trn_rl_repo")
import numpy as np

N_NODES = 50000
N_EDGES = 600000
H = 128
EPSILON = 0.7071067811865476
EPS2 = EPSILON * EPSILON
EPS = 1e-08
NCORES = 8
PERCORE = 6272          # 49 * 128 nodes per core
NBLK = 49               # node blocks per core
NBTOT = 392             # total node blocks (padded)
NPAD = 50176            # 392 * 128
CH = 4                  # edge tiles per DMA/gather chunk
ABATCH = 8              # phase-A node blocks per batch

SILU_NATIVE = True      # False: decompose silu into x*sigmoid(x) (for CoreSim)


def _preprocess(inputs):
    s = np.asarray(inputs["s"], np.float32).reshape(N_NODES, H)
    v = np.asarray(inputs["v"], np.float32).reshape(N_NODES, 3 * H)
    dir_ij = np.asarray(inputs["dir_ij"], np.float32)
    Wij = np.asarray(inputs["Wij"], np.float32).reshape(N_EDGES, 3 * H)
    senders = np.asarray(inputs["senders"]).astype(np.int64)
    receivers = np.asarray(inputs["receivers"]).astype(np.int64)

    s_pad = np.zeros((NPAD, H), np.float32)
    s_pad[:N_NODES] = s
    v_flat = np.zeros((NPAD, 3 * H), np.float32)
    v_flat[:N_NODES] = v

    owner = senders // PERCORE
    ls_all = senders - owner * PERCORE
    bb_all = ls_all // 128
    lp_all = ls_all % 128

    counts = np.zeros((NCORES, NBLK), np.int64)
    for c in range(NCORES):
        counts[c] = np.bincount(bb_all[owner == c], minlength=NBLK)
    tiles_b = (-(-counts // 128)).max(axis=0)
    tiles_b = (-(-tiles_b // CH) * CH).astype(np.int64)   # pad to chunk multiple
    tile_base = np.concatenate([[0], np.cumsum(tiles_b)])
    t_total = int(tile_base[-1])
    nch_tot = t_total // CH

    shared = {
        "s_pad": s_pad,
        "v_flat": v_flat,
        "Wi1": np.asarray(inputs["Wi1"], np.float32),
        "bi1": np.asarray(inputs["bi1"], np.float32).reshape(H, 1),
        "Wi2": np.asarray(inputs["Wi2"], np.float32),
        "bi2": np.asarray(inputs["bi2"], np.float32).reshape(1, 3 * H),
        "Wm1a": np.ascontiguousarray(
            np.asarray(inputs["Wm1"], np.float32)[:H] * EPSILON),
        "Wm1b": np.ascontiguousarray(
            np.asarray(inputs["Wm1"], np.float32)[H:]),
        "bm1": np.asarray(inputs["bm1"], np.float32).reshape(H, 1),
        "Wm2": np.asarray(inputs["Wm2"], np.float32),
        "bm2": np.asarray(inputs["bm2"], np.float32).reshape(1, 3 * H),
        "Wvm": np.asarray(inputs["Wvm"], np.float32) * EPSILON,
    }

    per_core = []
    for c in range(NCORES):
        sel = np.nonzero(owner == c)[0]
        bb = bb_all[sel]
        order = np.lexsort((receivers[sel], bb))
        sel = sel[order]
        bb = bb[order]
        cnt = np.bincount(bb, minlength=NBLK)
        src = np.full(t_total * 128, -1, np.int64)
        ofs = 0
        for b in range(NBLK):
            n = int(cnt[b])
            r0 = int(tile_base[b]) * 128
            src[r0:r0 + n] = np.arange(ofs, ofs + n)
            ofs += n
        mask = src >= 0
        gsel = sel[src[mask]]
        wij_s = np.zeros((t_total * 128, 3 * H), np.float32)
        wij_s[mask] = Wij[gsel]
        meta = np.zeros((t_total * 128, 4), np.float32)
        meta[:, 3] = 200.0
        meta[mask, 0:3] = dir_ij[gsel]
        meta[mask, 3] = lp_all[gsel]
        ridx = np.zeros((t_total * 128,), np.int32)
        ridx[mask] = receivers[gsel].astype(np.int32)
        per_core.append({
            "wij": wij_s.reshape(nch_tot, 128, CH, 3 * H),
            "meta": meta.reshape(nch_tot, 128, CH, 4),
            "ridx": ridx.reshape(nch_tot, 128, CH),
            "s_own": np.ascontiguousarray(s_pad[c * PERCORE:(c + 1) * PERCORE]),
            "v_own": np.ascontiguousarray(v_flat[c * PERCORE:(c + 1) * PERCORE]),
        })
    return shared, per_core, [int(x) for x in tiles_b], nch_tot


def _build(nc, tiles_b, nch_tot):
    from concourse import bass, tile, mybir
    from concourse.masks import make_identity

    F32 = mybir.dt.float32
    F32R = mybir.dt.float32r
    I32 = mybir.dt.int32
    AF = mybir.ActivationFunctionType
    OP = mybir.AluOpType

    def dt(name, shape, dtype=F32, kind="ExternalInput"):
        return nc.dram_tensor(name, shape, dtype, kind=kind).ap()

    s_pad_d = dt("s_pad", [NPAD, H])
    v_flat_d = dt("v_flat", [NPAD, 3 * H])
    wij_d = dt("wij", [nch_tot, 128, CH, 3 * H])
    meta_d = dt("meta", [nch_tot, 128, CH, 4])
    ridx_d = dt("ridx", [nch_tot, 128, CH], I32)
    s_own_d = dt("s_own", [PERCORE, H])
    v_own_d = dt("v_own", [PERCORE, 3 * H])
    wi1_d = dt("Wi1", [H, H])
    bi1_d = dt("bi1", [H, 1])
    wi2_d = dt("Wi2", [H, 3 * H])
    bi2_d = dt("bi2", [1, 3 * H])
    wm1a_d = dt("Wm1a", [H, H])
    wm1b_d = dt("Wm1b", [H, H])
    bm1_d = dt("bm1", [H, 1])
    wm2_d = dt("Wm2", [H, 3 * H])
    bm2_d = dt("bm2", [1, 3 * H])
    wvm_d = dt("Wvm", [H, 2 * H])
    x_tab = dt("x_tab", [NPAD, 3 * H], kind="Internal")
    out_d = dt("out", [PERCORE, 4 * H], kind="ExternalOutput")

    with tile.TileContext(nc) as tc:
        with tc.tile_pool(name="const", bufs=1) as cp:
            ident = cp.tile([128, 128], F32, name="ident")
            make_identity(nc, ident[:])
            iota_i = cp.tile([128, 128], I32, name="iota_i")
            nc.gpsimd.iota(iota_i[:], pattern=[[1, 128]], base=0,
                           channel_multiplier=0)
            iota_f = cp.tile([128, 128], F32, name="iota_f")
            nc.vector.tensor_copy(out=iota_f[:], in_=iota_i[:])
            eps_t = cp.tile([128, 1], F32, name="eps_t")
            nc.vector.memset(eps_t[:], EPS)
            epsl_t = cp.tile([128, 1], F32, name="epsl_t")
            nc.vector.memset(epsl_t[:], EPSILON)
            ones1 = cp.tile([1, 128], F32, name="ones1")
            nc.vector.memset(ones1[:], 1.0)

            def load(name, dram, shape):
                t = cp.tile(shape, F32, name=name)
                nc.sync.dma_start(out=t[:], in_=dram[:])
                return t

            wi1_t = load("wi1_t", wi1_d, [H, H])
            bi1_t = load("bi1_t", bi1_d, [H, 1])
            wi2_t = load("wi2_t", wi2_d, [H, 3 * H])
            bi2_t = load("bi2_t", bi2_d, [1, 3 * H])
            wm1a_t = load("wm1a_t", wm1a_d, [H, H])
            wm1b_t = load("wm1b_t", wm1b_d, [H, H])
            bm1_t = load("bm1_t", bm1_d, [H, 1])
            wm2_t = load("wm2_t", wm2_d, [H, 3 * H])
            bm2_t = load("bm2_t", bm2_d, [1, 3 * H])
            wvm_t = load("wvm_t", wvm_d, [H, 2 * H])

            def silu(pool, out_ap, in_ps_ap, bias_ap, shape, tag):
                if SILU_NATIVE:
                    nc.scalar.activation(out=out_ap, in_=in_ps_ap, func=AF.Silu,
                                         bias=bias_ap)
                else:
                    z = pool.tile(shape, F32, name=tag + "_z")
                    nc.vector.tensor_scalar(out=z[:], in0=in_ps_ap,
                                            scalar1=bias_ap, scalar2=None,
                                            op0=OP.add)
                    sg = pool.tile(shape, F32, name=tag + "_sg")
                    nc.scalar.activation(out=sg[:], in_=in_ps_ap,
                                         func=AF.Sigmoid, bias=bias_ap)
                    nc.vector.tensor_tensor(out=out_ap, in0=z[:], in1=sg[:],
                                            op=OP.mult)

            # ---------------- Phase A: x_tab = MLP_i(s) for all nodes -------
            with tc.tile_pool(name="pa", bufs=2) as pa, \
                 tc.tile_pool(name="psa", bufs=2, space="PSUM") as psa:
                for g in range(NBTOT // ABATCH):
                    r0 = g * ABATCH * 128
                    r1 = r0 + ABATCH * 128
                    s8 = pa.tile([128, ABATCH, H], F32, name="s8")
                    nc.sync.dma_start(
                        out=s8[:],
                        in_=s_pad_d[r0:r1, :].rearrange("(j p) f -> p j f",
                                                        p=128))
                    x8 = pa.tile([128, ABATCH, 3 * H], F32, name="x8")
                    for half in range(ABATCH // 4):
                        sT4 = pa.tile([128, 512], F32, name="sT4", bufs=4)
                        for jj in range(4):
                            j = half * 4 + jj
                            trp = psa.tile([128, 128], F32, name="trp")
                            nc.tensor.transpose(trp[:],
                                                in_=s8[:, j, :],
                                                identity=ident[:])
                            nc.scalar.activation(
                                out=sT4[:, jj * 128:(jj + 1) * 128],
                                in_=trp[:], func=AF.Copy)
                        h4 = psa.tile([128, 512], F32, name="h4")
                        nc.tensor.matmul(h4[:], lhsT=wi1_t[:],
                                         rhs=sT4[:],
                                         start=True, stop=True)
                        h4s = pa.tile([128, 512], F32, name="h4s", bufs=4)
                        silu(pa, h4s[:], h4[:], bi1_t[:], [128, 512], "sa")
                        for jj in range(4):
                            j = half * 4 + jj
                            xps = psa.tile([128, 3 * H], F32, name="xps")
                            nc.tensor.matmul(xps[:],
                                             lhsT=ones1[:],
                                             rhs=bi2_t[:],
                                             start=True, stop=False)
                            nc.tensor.matmul(
                                xps[:],
                                lhsT=h4s[:, jj * 128:(jj + 1) * 128]
                                ,
                                rhs=wi2_t[:],
                                start=False, stop=True)
                            nc.vector.tensor_copy(out=x8[:, j, :], in_=xps[:])
                    nc.sync.dma_start(
                        out=x_tab[r0:r1, :].rearrange("(j p) f -> p j f",
                                                      p=128),
                        in_=x8[:])

            # ------------- Phase B + C: messages, scatter, update -----------
            with tc.tile_pool(name="pb", bufs=3) as pb, \
                 tc.tile_pool(name="pc", bufs=2) as pcp, \
                 tc.tile_pool(name="psb", bufs=2, space="PSUM") as psb, \
                 tc.tile_pool(name="psc", bufs=1, space="PSUM") as psc:
                ch_cursor = 0
                for b in range(NBLK):
                    ntiles = tiles_b[b]
                    nch = ntiles // CH
                    pblk = psb.tile([128, 512], F32, name="pblk")
                    for ci in range(nch):
                        gci = ch_cursor + ci
                        wij_t = pb.tile([128, CH, 3 * H], F32, name="wij_t")
                        nc.sync.dma_start(out=wij_t[:], in_=wij_d[gci])
                        meta_t = pb.tile([128, CH, 4], F32, name="meta_t")
                        nc.sync.dma_start(out=meta_t[:], in_=meta_d[gci])
                        ridx_t = pb.tile([128, CH], I32, name="ridx_t")
                        nc.sync.dma_start(out=ridx_t[:], in_=ridx_d[gci])
                        xj = pb.tile([128, CH, 3 * H], F32, name="xj")
                        vj = pb.tile([128, CH, 3 * H], F32, name="vj")
                        for j in range(CH):
                            nc.gpsimd.indirect_dma_start(
                                out=xj[:, j, :], out_offset=None, in_=x_tab[:],
                                in_offset=bass.IndirectOffsetOnAxis(
                                    ap=ridx_t[:, j:j + 1], axis=0))
                            nc.gpsimd.indirect_dma_start(
                                out=vj[:, j, :], out_offset=None,
                                in_=v_flat_d[:],
                                in_offset=bass.IndirectOffsetOnAxis(
                                    ap=ridx_t[:, j:j + 1], axis=0))
                        for j in range(CH):
                            t = ci * CH + j
                            msg = pb.tile([128, 512], F32, name="msg", bufs=6)
                            nc.gpsimd.tensor_tensor(
                                out=msg[:, 0:128], in0=wij_t[:, j, 0:128],
                                in1=xj[:, j, 0:128], op=OP.mult)
                            prod23 = pb.tile([128, 256], F32, name="prod23",
                                             bufs=6)
                            nc.vector.tensor_tensor(
                                out=prod23[:], in0=wij_t[:, j, 128:384],
                                in1=xj[:, j, 128:384], op=OP.mult)
                            tmp = pb.tile([128, 384], F32, name="tmp", bufs=6)
                            parts = pb.tile([128, 384], F32, name="parts",
                                            bufs=6)
                            for d in range(3):
                                nc.scalar.activation(
                                    out=parts[:, d * 128:(d + 1) * 128],
                                    in_=prod23[:, 0:128], func=AF.Copy,
                                    scale=meta_t[:, j, d:d + 1])
                                nc.vector.tensor_tensor(
                                    out=tmp[:, d * 128:(d + 1) * 128],
                                    in0=prod23[:, 128:256],
                                    in1=vj[:, j, d * 128:(d + 1) * 128],
                                    op=OP.mult)
                            nc.gpsimd.tensor_tensor(out=msg[:, 128:512],
                                                    in0=parts[:], in1=tmp[:],
                                                    op=OP.add)
                            oh = pb.tile([128, 128], F32, name="oh", bufs=6)
                            nc.gpsimd.tensor_scalar(
                                out=oh[:], in0=iota_f[:],
                                scalar1=meta_t[:, j, 3:4], scalar2=None,
                                op0=OP.is_equal)
                            nc.tensor.matmul(pblk[:],
                                             lhsT=oh[:],
                                             rhs=msg[:],
                                             start=(t == 0),
                                             stop=(t == ntiles - 1))
                    ch_cursor += nch
                    # ---- Phase C for block b ----
                    s_own_t = pcp.tile([128, H], F32, name="s_own_t")
                    nc.sync.dma_start(out=s_own_t[:],
                                      in_=s_own_d[b * 128:(b + 1) * 128, :])
                    v_own_t = pcp.tile([128, 3 * H], F32, name="v_own_t")
                    nc.sync.dma_start(out=v_own_t[:],
                                      in_=v_own_d[b * 128:(b + 1) * 128, :])
                    sv_raw = pcp.tile([128, H], F32, name="sv_raw")
                    vsum = pcp.tile([128, 3 * H], F32, name="vsum")
                    if ntiles > 0:
                        nc.vector.tensor_tensor(out=sv_raw[:], in0=s_own_t[:],
                                                in1=pblk[:, 0:128], op=OP.add)
                        nc.vector.tensor_tensor(out=vsum[:], in0=v_own_t[:],
                                                in1=pblk[:, 128:512],
                                                op=OP.add)
                    else:
                        nc.vector.tensor_copy(out=sv_raw[:], in_=s_own_t[:])
                        nc.vector.tensor_copy(out=vsum[:], in_=v_own_t[:])
                    vwd = []
                    sq = []
                    for d in range(3):
                        trc = psc.tile([128, 128], F32, name="trc", bufs=1)
                        nc.tensor.transpose(
                            trc[:],
                            in_=vsum[:, d * 128:(d + 1) * 128],
                            identity=ident[:])
                        vT = pcp.tile([128, 128], F32, name="vT", bufs=6)
                        nc.scalar.activation(out=vT[:], in_=trc[:],
                                             func=AF.Copy)
                        vw = psc.tile([128, 2 * H], F32, name="vw", bufs=3)
                        nc.tensor.matmul(vw[:], lhsT=vT[:],
                                         rhs=wvm_t[:],
                                         start=True, stop=True)
                        vws = pcp.tile([128, 2 * H], F32, name="vws", bufs=6)
                        nc.scalar.activation(out=vws[:], in_=vw[:],
                                             func=AF.Copy)
                        vwd.append(vws)
                        sq_d = pcp.tile([128, 128], F32, name="sq", bufs=6)
                        nc.vector.tensor_tensor(out=sq_d[:],
                                                in0=vws[:, 128:256],
                                                in1=vws[:, 128:256],
                                                op=OP.mult)
                        sq.append(sq_d)
                    acc = pcp.tile([128, 128], F32, name="acc")
                    nc.vector.tensor_tensor(out=acc[:], in0=sq[0][:],
                                            in1=sq[1][:], op=OP.add)
                    nc.vector.tensor_tensor(out=acc[:], in0=acc[:],
                                            in1=sq[2][:], op=OP.add)
                    vnorm = pcp.tile([128, 128], F32, name="vnorm")
                    nc.scalar.activation(out=vnorm[:], in_=acc[:],
                                         func=AF.Sqrt, bias=eps_t[:])
                    hps = psc.tile([128, 128], F32, name="hps")
                    for k, src_t in enumerate((sv_raw, vnorm)):
                        trc = psc.tile([128, 128], F32, name="trc", bufs=1)
                        nc.tensor.transpose(trc[:],
                                            in_=src_t[:],
                                            identity=ident[:])
                        tsT = pcp.tile([128, 128], F32, name="tsT", bufs=4)
                        nc.scalar.activation(out=tsT[:], in_=trc[:],
                                             func=AF.Copy)
                        lhs = wm1a_t if k == 0 else wm1b_t
                        nc.tensor.matmul(hps[:], lhsT=lhs[:],
                                         rhs=tsT[:],
                                         start=(k == 0), stop=(k == 1))
                    hsb = pcp.tile([128, 128], F32, name="hsb")
                    silu(pcp, hsb[:], hps[:], bm1_t[:], [128, 128], "sc")
                    ops_ = psc.tile([128, 3 * H], F32, name="ops")
                    nc.tensor.matmul(ops_[:], lhsT=ones1[:],
                                     rhs=bm2_t[:],
                                     start=True, stop=False)
                    nc.tensor.matmul(ops_[:], lhsT=hsb[:],
                                     rhs=wm2_t[:],
                                     start=False, stop=True)
                    svl = pcp.tile([128, 128], F32, name="svl")
                    m2 = pcp.tile([128, 128], F32, name="m2")
                    nc.vector.tensor_tensor(out=svl[:], in0=vwd[0][:, 0:128],
                                            in1=vwd[0][:, 128:256],
                                            op=OP.mult)
                    nc.vector.tensor_tensor(out=m2[:], in0=vwd[1][:, 0:128],
                                            in1=vwd[1][:, 128:256],
                                            op=OP.mult)
                    nc.vector.tensor_tensor(out=svl[:], in0=svl[:], in1=m2[:],
                                            op=OP.add)
                    nc.vector.tensor_tensor(out=m2[:], in0=vwd[2][:, 0:128],
                                            in1=vwd[2][:, 128:256],
                                            op=OP.mult)
                    nc.vector.tensor_tensor(out=svl[:], in0=svl[:], in1=m2[:],
                                            op=OP.add)
                    dsv = pcp.tile([128, 128], F32, name="dsv")
                    nc.vector.tensor_tensor(out=dsv[:], in0=ops_[:, 256:384],
                                            in1=svl[:], op=OP.mult)
                    accs = pcp.tile([128, 128], F32, name="accs")
                    nc.vector.tensor_tensor(out=accs[:], in0=ops_[:, 0:128],
                                            in1=dsv[:], op=OP.add)
                    outt = pcp.tile([128, 4 * H], F32, name="outt")
                    t1 = pcp.tile([128, 128], F32, name="t1")
                    nc.scalar.activation(out=t1[:], in_=accs[:], func=AF.Copy,
                                         scale=epsl_t[:])
                    t2 = pcp.tile([128, 128], F32, name="t2")
                    nc.gpsimd.tensor_scalar(out=t2[:], in0=sv_raw[:],
                                            scalar1=EPS2, scalar2=None,
                                            op0=OP.mult)
                    nc.vector.tensor_tensor(out=outt[:, 0:128], in0=t1[:],
                                            in1=t2[:], op=OP.add)
                    for d in range(3):
                        q = pcp.tile([128, 128], F32, name="qd", bufs=6)
                        nc.vector.tensor_tensor(out=q[:],
                                                in0=vwd[d][:, 0:128],
                                                in1=ops_[:, 128:256],
                                                op=OP.mult)
                        qs = pcp.tile([128, 128], F32, name="qsd", bufs=6)
                        nc.scalar.activation(out=qs[:], in_=q[:], func=AF.Copy,
                                             scale=epsl_t[:])
                        r = pcp.tile([128, 128], F32, name="rd", bufs=6)
                        nc.gpsimd.tensor_scalar(
                            out=r[:], in0=vsum[:, d * 128:(d + 1) * 128],
                            scalar1=EPS2, scalar2=None, op0=OP.mult)
                        nc.vector.tensor_tensor(
                            out=outt[:, 128 + d * 128:256 + d * 128],
                            in0=qs[:], in1=r[:], op=OP.add)
                    nc.sync.dma_start(out=out_d[b * 128:(b + 1) * 128, :],
                                      in_=outt[:])


def _install_trace_hook():
    try:
        import antenv
        if "antenv.axon_hooks" not in sys.modules:
            mod = types.ModuleType("antenv.axon_hooks")
            mod._hook = None

            def set_axon_ntff_profile_hook(h):
                mod._hook = h

            def get_axon_ntff_profile_hook():
                return mod._hook

            mod.set_axon_ntff_profile_hook = set_axon_ntff_profile_hook
            mod.get_axon_ntff_profile_hook = get_axon_ntff_profile_hook
            sys.modules["antenv.axon_hooks"] = mod
            antenv.axon_hooks = mod
        from antenv.axon_hooks import (get_axon_ntff_profile_hook,
                                       set_axon_ntff_profile_hook)
        if get_axon_ntff_profile_hook() is None:
            from trn_agent_boot.trn_boot import _ntff_profile_via_ctypes
            set_axon_ntff_profile_hook(
                _ntff_profile_via_ctypes("/opt/axon/libaxon_pjrt.so"))
        return True
    except Exception:
        return False


def kernel(**inputs):
    from concourse import bacc
    from concourse.bass_utils import run_bass_kernel_spmd

    shared, per_core, tiles_b, nch_tot = _preprocess(inputs)
    nc = bacc.Bacc("TRN2", target_bir_lowering=False, debug=False,
                   num_devices=NCORES)
    _build(nc, tiles_b, nch_tot)
    nc.compile()

    in_maps = [dict(shared, **per_core[c]) for c in range(NCORES)]
    trace = _install_trace_hook()
    try:
        res = run_bass_kernel_spmd(nc, in_maps, core_ids=list(range(NCORES)),
                                   trace=trace)
    except Exception:
        if not trace:
            raise
        res = run_bass_kernel_spmd(nc, in_maps, core_ids=list(range(NCORES)),
                                   trace=False)
    kernel.last_exec_time_ns = getattr(res, "exec_time_ns", None)
    outs = [np.asarray(res.results[c]["out"]) for c in range(NCORES)]
    full = np.concatenate(outs, axis=0)[:N_NODES]
    return np.ascontiguousarray(full.reshape(N_NODES, 4, H), dtype=np.float32)
